# revision 23
# baseline (speedup 1.0000x reference)
"""Causal single-head attention (B=4, S=2048, D=1024, fp32) on 8 TRN2 NeuronCores.

Sharding: core c <-> (batch c//2, parity c%2). Each core owns the 8 even or
odd 128-row query tiles of its batch (balanced causal work, uniform extents
2j+2 with a 2-tile parity mask). Q is projected locally. The K/V projections
are split across the pair: parity 0 computes K^T for all 2048 keys, parity 1
computes V — the *same* instruction stream on both (stationary [128,128]
blocks x moving [128,1024]), with x and W in swapped roles via the packed
PSTAT/PMOV inputs. Products are exchanged per 1024-key pass as bf16 blobs
through a pairwise DRAM AllGather, then flash-style causal attention runs
over the pass's keys (scores/AV in bf16, accum fp32, partial O carried in
SBUF between passes).

Self-contained: hardcodes shapes; reads nothing from disk.
"""
import sys

import numpy as np

try:
    from concourse import bass, bacc, tile
except ImportError:  # concourse ships with the container, not this file
    for _p in ("/opt/trn_rl_repo", "/root/.axon_site/_ro/trn_rl_repo"):
        if _p not in sys.path:
            sys.path.append(_p)
    from concourse import bass, bacc, tile
from concourse import mybir
from concourse.bass_utils import run_bass_kernel_spmd

dt = mybir.dt
AF = mybir.ActivationFunctionType

B, S, D = 4, 2048, 1024
P = 128
ND = D // P          # 8 d-tiles (contraction of projections)
NO = 8               # out tiles per projection pass (e-tiles or s-tiles)
SLOTS = 8            # q-tiles per core
HT = 8               # k-tiles per pass
NCORES = 8
SCALE = 1.0 / float(np.sqrt(D))
NEG = -1.0e30
GROUPS = [[0, 1], [2, 3], [4, 5], [6, 7]]
SIM_LOCAL_CC = False  # replace collectives with local DMAs (TimelineSim only)
NO_CC = False         # timing probe: skip exchange, read own blob (wrong data)
# slot j of parity r holds query tile 2j+r; uniform program extent 2j+2 k-tiles
# with a parity-dependent 2-tile mask finishing the causal boundary


def _emit_body(nc, tc, pools, aps, rep):
    (sb_const, sb_xq, sb_qt, sb_wblk, sb_wq, sb_wmov, sb_stage, sb_ofin,
     sb_p, sb_pt, sb_sums, sb_obuf, sb_ojoin, dram, ps) = pools
    xqT, Wq, PSTAT, PMOV, mask, ident, O = aps

    ident_b = sb_const.tile([P, P], dt.bfloat16, tag="ident", name="ident_b")
    nc.sync.dma_start(ident_b[:], ident[:])
    mask_sb = sb_const.tile([P, 2 * P], dt.float32, tag="mask", name="mask_sb")
    nc.sync.dma_start(mask_sb[:], mask[:])

    # ---------- K-or-V projection per pass + pairwise exchange ----------
    bounce_in = [dram.tile([P, NO * D], dt.bfloat16, tag=f"bin{p}",
                           name=f"bin{p}_{rep}") for p in range(2)]
    bounce_out = [dram.tile([2 * P, NO * D], dt.bfloat16, tag=f"bout{p}",
                            name=f"bout{p}_{rep}") for p in range(2)]
    # prefetch both passes' moving operands + pass-0 stationaries (sync queue);
    # xq/Wq ride the scalar HWDGE queue so they load in parallel
    wmovs = [sb_wmov.tile([P, ND * D], dt.bfloat16, tag="wmov", name=f"wmov{p}")
             for p in range(2)]
    for q in range(4):
        nc.sync.dma_start(wmovs[0][:, q * 2048:(q + 1) * 2048],
                          PMOV[0:P, q * 2048:(q + 1) * 2048])
    wsts = [None] * (2 * NO)
    for o in range(NO):
        wst = sb_wblk.tile([P, ND * P], dt.bfloat16, tag=f"kv{o}", name="wst")
        nc.sync.dma_start(wst[:], PSTAT[0:P, o * 1024:(o + 1) * 1024])
        wsts[o] = wst
    for q in range(4):
        nc.sync.dma_start(wmovs[1][:, q * 2048:(q + 1) * 2048],
                          PMOV[0:P, 8192 + q * 2048:8192 + (q + 1) * 2048])
    xq = [sb_xq.tile([P, SLOTS * P], dt.bfloat16, tag=f"xq{d}", name=f"xq{d}")
          for d in range(ND)]
    for d in range(ND):
        nc.scalar.dma_start(xq[d][:], xqT[d * P:(d + 1) * P, :])
    wbs = []
    for e in range(NO):
        wb = sb_wq.tile([P, ND * P], dt.bfloat16, tag=f"wq{e}", name="wb")
        nc.scalar.dma_start(wb[:], Wq[e * P:(e + 1) * P, :])
        wbs.append(wb)

    for p in range(2):
        wmov = wmovs[p]
        for o in range(NO):
            wst = wsts[p * NO + o]
            pr0 = ps.tile([P, 512], dt.float32, tag="proj", name="pr0")
            pr1 = ps.tile([P, 512], dt.float32, tag="proj", name="pr1")
            for d in range(ND):
                nc.tensor.matmul(pr0[:], wst[:, d * P:(d + 1) * P],
                                 wmov[:, d * D:d * D + 512],
                                 start=(d == 0), stop=(d == ND - 1))
                nc.tensor.matmul(pr1[:], wst[:, d * P:(d + 1) * P],
                                 wmov[:, d * D + 512:(d + 1) * D],
                                 start=(d == 0), stop=(d == ND - 1))
            stg = sb_stage.tile([P, D], dt.bfloat16, tag="stage", name="stg")
            nc.vector.tensor_copy(stg[:, 0:512], pr0[:])
            nc.vector.tensor_copy(stg[:, 512:1024], pr1[:])
            nc.sync.dma_start(bounce_in[p][:, o * D:(o + 1) * D], stg[:])
            if p == 0:  # rotate this stationary buffer to its pass-1 slice
                wst1 = sb_wblk.tile([P, ND * P], dt.bfloat16, tag=f"kv{o}", name="wst")
                nc.sync.dma_start(wst1[:], PSTAT[0:P, 8192 + o * 1024:8192 + (o + 1) * 1024])
                wsts[NO + o] = wst1
        if NO_CC:
            pass
        elif SIM_LOCAL_CC:
            nc.gpsimd.dma_start(bounce_out[p][0:P, :], bounce_in[p][:])
            nc.gpsimd.dma_start(bounce_out[p][P:2 * P, :], bounce_in[p][:])
        else:
            nc.gpsimd.collective_compute(
                "AllGather", mybir.AluOpType.bypass, replica_groups=GROUPS,
                ins=[bounce_in[p][:].opt()], outs=[bounce_out[p][:].opt()])

    # ---------- Q^T projection (own queries), bf16 out ----------
    qt = [sb_qt.tile([P, SLOTS * P], dt.bfloat16, tag=f"qt{e}", name=f"qt{e}")
          for e in range(NO)]
    for e in range(NO):
        wb = wbs[e]
        qp0 = ps.tile([P, 512], dt.float32, tag="proj", name="qp0")
        qp1 = ps.tile([P, 512], dt.float32, tag="proj", name="qp1")
        for d in range(ND):
            nc.tensor.matmul(qp0[:], wb[:, d * P:(d + 1) * P], xq[d][:, 0:512],
                             start=(d == 0), stop=(d == ND - 1))
            nc.tensor.matmul(qp1[:], wb[:, d * P:(d + 1) * P], xq[d][:, 512:1024],
                             start=(d == 0), stop=(d == ND - 1))
        nc.vector.tensor_copy(qt[e][:, 0:512], qp0[:])
        nc.vector.tensor_copy(qt[e][:, 512:1024], qp1[:])

    # per-slot softmax chunk sums + norm scratch (cols 4j..4j+3 sums; 32+2j ssum; 33+2j rec)
    sums = sb_sums.tile([P, 48], dt.float32, tag="sums", name="sums")
    obuf = [sb_obuf.tile([P, D], dt.float32, tag=f"ob{j}", name=f"ob{j}")
            for j in range(4)]          # carry for slots 4..7
    chunk_ctr = [0] * SLOTS

    for p in range(2):
        # kt/vt for this pass from the exchanged blobs, recycling wmov buffers
        kv_k = sb_wmov.tile([P, NO * D], dt.bfloat16, tag="wmov", name=f"kvk{p}")
        kv_v = sb_wmov.tile([P, NO * D], dt.bfloat16, tag="wmov", name=f"kvv{p}")
        ksrc = bounce_in[p] if NO_CC else bounce_out[p][0:P, :]
        vsrc = bounce_in[p] if NO_CC else bounce_out[p][P:2 * P, :]
        for h in range(2):
            nc.sync.dma_start(kv_k[:, h * 4096:(h + 1) * 4096],
                              ksrc[:, h * 4096:(h + 1) * 4096])
        for h in range(2):
            nc.sync.dma_start(kv_v[:, h * 4096:(h + 1) * 4096],
                              vsrc[:, h * 4096:(h + 1) * 4096])

        for j in range(SLOTS):
            ext = 2 * j + 2                     # uniform extent in k-tiles
            t0 = HT * p
            t1 = min(ext, HT * (p + 1))
            if t1 <= t0:
                continue
            final_pass = (ext - 1) // HT == p

            o_ps = [ps.tile([P, 512], dt.float32, tag="o", name=f"ops{ec}")
                    for ec in range(2)]

            c = t0 * P
            while c < t1 * P:
                cw = 512 if t1 * P - c >= 512 else t1 * P - c
                sps = ps.tile([P, cw], dt.float32, tag="s", name="sps")
                for e in range(NO):
                    nc.tensor.matmul(sps[:], qt[e][:, j * P:(j + 1) * P],
                                     kv_k[:, e * D + c - p * 1024: e * D + c - p * 1024 + cw],
                                     start=(e == 0), stop=(e == NO - 1))
                if final_pass and c + cw == ext * P:
                    off = cw - 2 * P
                    nc.vector.tensor_add(sps[:, off:off + 2 * P],
                                         sps[:, off:off + 2 * P], mask_sb[:])
                pch = sb_p.tile([P, cw], dt.bfloat16, tag="p", name="pch")
                ci = chunk_ctr[j]
                chunk_ctr[j] += 1
                nc.scalar.activation(pch[:], sps[:], AF.Exp, scale=SCALE,
                                     accum_out=sums[:, 4 * j + ci:4 * j + ci + 1])
                for ti in range(cw // P):
                    t = c // P + ti
                    ptb = sb_pt.tile([P, P], dt.bfloat16, tag="pt", name="ptb")
                    nc.sync.dma_start(ptb[:], pch[:, ti * P:(ti + 1) * P], transpose=True)
                    for ec in range(2):
                        nc.tensor.matmul(o_ps[ec][:], ptb[:],
                                         kv_v[:, (t - HT * p) * D + ec * 512:(t - HT * p) * D + (ec + 1) * 512],
                                         start=(t == t0), stop=(t == t1 - 1))
                c += cw

            if final_pass:
                ssum = sums[:, 32 + 2 * j:33 + 2 * j]
                nc.vector.reduce_sum(ssum, sums[:, 4 * j:4 * j + chunk_ctr[j]],
                                     axis=mybir.AxisListType.X)
                rec = sums[:, 33 + 2 * j:34 + 2 * j]
                nc.vector.reciprocal(rec, ssum)
                ob = sb_ofin.tile([P, D], dt.float32, tag="obf", name="obfin")
                for ec in range(2):
                    if ext > HT:
                        oj = sb_ojoin.tile([P, 512], dt.float32, tag="ojoin", name="oj")
                        nc.vector.tensor_add(oj[:], o_ps[ec][:],
                                             obuf[j - 4][:, ec * 512:(ec + 1) * 512])
                        nc.scalar.mul(ob[:, ec * 512:(ec + 1) * 512], oj[:], rec)
                    else:
                        nc.scalar.mul(ob[:, ec * 512:(ec + 1) * 512], o_ps[ec][:], rec)
                nc.sync.dma_start(O[j * P:(j + 1) * P, :], ob[:])
            else:
                for ec in range(2):
                    nc.vector.tensor_copy(obuf[j - 4][:, ec * 512:(ec + 1) * 512],
                                          o_ps[ec][:])


def build_program(reps: int = 1):
    nc = bacc.Bacc("TRN2", target_bir_lowering=False, debug=False, num_devices=NCORES)

    xqT_t = nc.dram_tensor("xqT", [D, SLOTS * P], dt.bfloat16, kind="ExternalInput")
    Wq_t = nc.dram_tensor("Wq", [D, D], dt.bfloat16, kind="ExternalInput")
    PSTAT_t = nc.dram_tensor("PSTAT", [P, 2 * NO * ND * P], dt.bfloat16, kind="ExternalInput")
    PMOV_t = nc.dram_tensor("PMOV", [P, 2 * ND * D], dt.bfloat16, kind="ExternalInput")
    mask_t = nc.dram_tensor("mask", [P, 2 * P], dt.float32, kind="ExternalInput")
    ident_t = nc.dram_tensor("ident", [P, P], dt.bfloat16, kind="ExternalInput")
    O_t = nc.dram_tensor("O", [SLOTS * P, D], dt.float32, kind="ExternalOutput")

    with tile.TileContext(nc) as tc:
        with (
            tc.tile_pool(name="const", bufs=1) as sb_const,
            tc.tile_pool(name="xq", bufs=1) as sb_xq,
            tc.tile_pool(name="qt", bufs=1) as sb_qt,
            tc.tile_pool(name="wstat", bufs=2) as sb_wblk,
            tc.tile_pool(name="wq", bufs=1) as sb_wq,
            tc.tile_pool(name="wmov", bufs=4) as sb_wmov,
            tc.tile_pool(name="stage", bufs=3) as sb_stage,
            tc.tile_pool(name="ofin", bufs=2) as sb_ofin,
            tc.tile_pool(name="p", bufs=3) as sb_p,
            tc.tile_pool(name="pt", bufs=10) as sb_pt,
            tc.tile_pool(name="sums", bufs=1) as sb_sums,
            tc.tile_pool(name="obuf", bufs=1) as sb_obuf,
            tc.tile_pool(name="ojoin", bufs=2) as sb_ojoin,
            tc.tile_pool(name="dram", bufs=2, space=bass.MemorySpace.DRAM) as dram,
            tc.tile_pool(name="ps", bufs=2, space=bass.MemorySpace.PSUM) as ps,
        ):
            pools = (sb_const, sb_xq, sb_qt, sb_wblk, sb_wq, sb_wmov,
                     sb_stage, sb_ofin, sb_p, sb_pt, sb_sums, sb_obuf, sb_ojoin,
                     dram, ps)
            # reps are unrolled (python loop, NOT tc.For_i): collectives are
            # not supported inside hardware loops on this runtime path.
            for r in range(reps):
                aps = (xqT_t.ap(), Wq_t.ap(), PSTAT_t.ap(), PMOV_t.ap(),
                       mask_t.ap(), ident_t.ap(), O_t.ap())
                _emit_body(nc, tc, pools, aps, r)

    nc.compile()
    return nc


def round_f32r(a):
    """Round fp32 to the fp32r grid (low 12 mantissa bits dropped, nearest-even)."""
    u = np.ascontiguousarray(a, np.float32).view(np.uint32).copy()
    low = u & np.uint32(0xFFF)
    base = u & np.uint32(0xFFFFF000)
    up = (low > 0x800) | ((low == 0x800) & (((base >> 12) & 1) == 1))
    base[up] += np.uint32(0x1000)
    return base.view(np.float32)


def pack_w_blocks(W):
    """[D, D] -> [D, D]: row (e*P+p), col (d*P+c) = W[d*P+p, e*P+c]."""
    return np.ascontiguousarray(
        W.reshape(ND, P, NO, P).transpose(2, 1, 0, 3).reshape(D, D))


def make_in_maps(x, Wq, Wk, Wv):
    import ml_dtypes
    x = round_f32r(x.reshape(B, S, D))
    Wq_p = pack_w_blocks(round_f32r(Wq)).astype(ml_dtypes.bfloat16)
    Wk_p = pack_w_blocks(round_f32r(Wk))
    Wv_r = round_f32r(Wv)
    ident = np.eye(P, dtype=np.float32).astype(ml_dtypes.bfloat16)
    tri = np.where(np.arange(P)[None, :] <= np.arange(P)[:, None], 0.0, NEG).astype(np.float32)
    masks = [
        np.concatenate([tri, np.full((P, P), NEG, np.float32)], axis=1),   # parity 0
        np.concatenate([np.zeros((P, P), np.float32), tri], axis=1),       # parity 1
    ]
    xT = [np.ascontiguousarray(x[b].T) for b in range(B)]   # [D, S] per batch

    # K-core PSTAT: pack[o*P+p, d*P+c] -> [p, o*1024 + d*128 + c], same both passes
    pk = Wk_p.reshape(NO, P, ND * P).transpose(1, 0, 2).reshape(P, NO * ND * P)
    PSTAT_K = np.ascontiguousarray(np.concatenate([pk, pk], axis=1)).astype(ml_dtypes.bfloat16)
    # V-core PMOV: Wv[d*P+p, c] -> [p, d*1024 + c], same both passes
    pv = Wv_r.reshape(ND, P, D).transpose(1, 0, 2).reshape(P, ND * D)
    PMOV_V = np.ascontiguousarray(np.concatenate([pv, pv], axis=1)).astype(ml_dtypes.bfloat16)

    in_maps = []
    for c in range(NCORES):
        b, r = c // 2, c % 2
        xTb = xT[b]
        cols = np.concatenate([np.arange((2 * j + r) * P, (2 * j + r + 1) * P)
                               for j in range(SLOTS)])
        xqTb = np.ascontiguousarray(xTb[:, cols]).astype(ml_dtypes.bfloat16)
        if r == 0:
            pstat = PSTAT_K
            # PMOV: xT[d*P+p, pass*1024 + c] -> [p, pass*8192 + d*1024 + c]
            pm = xTb.reshape(ND, P, 2, D).transpose(1, 2, 0, 3)
            pmov = np.ascontiguousarray(pm.reshape(P, 2 * ND * D)).astype(ml_dtypes.bfloat16)
        else:
            # PSTAT: xT[d*P+p, pass*1024 + o*128 + c] -> [p, pass*8192 + o*1024 + d*128 + c]
            t4 = xTb.reshape(ND, P, 2, NO, P).transpose(1, 2, 3, 0, 4)
            pstat = np.ascontiguousarray(t4.reshape(P, 2 * NO * ND * P)).astype(ml_dtypes.bfloat16)
            pmov = PMOV_V
        in_maps.append({
            "xqT": xqTb, "Wq": Wq_p, "PSTAT": pstat, "PMOV": pmov,
            "mask": masks[r], "ident": ident,
        })
    return in_maps


def assemble_output(results):
    out = np.empty((B, S, D), dtype=np.float32)
    for c in range(NCORES):
        b, r = c // 2, c % 2
        oc = results[c]["O"].reshape(SLOTS, P, D)
        for j in range(SLOTS):
            out[b, (2 * j + r) * P:(2 * j + r + 1) * P, :] = oc[j]
    return out


_nc_cache = {}


def _get_program(reps: int = 1):
    if reps not in _nc_cache:
        _nc_cache[reps] = build_program(reps)
    return _nc_cache[reps]


def kernel(x, Wq, Wk, Wv):
    x = np.asarray(x, dtype=np.float32)
    Wq = np.asarray(Wq, dtype=np.float32)
    Wk = np.asarray(Wk, dtype=np.float32)
    Wv = np.asarray(Wv, dtype=np.float32)
    nc = _get_program(1)
    in_maps = make_in_maps(x, Wq, Wk, Wv)
    results = run_bass_kernel_spmd(nc, in_maps, list(range(NCORES))).results
    return assemble_output(results)


# revision 24
# speedup vs baseline: 1.2249x; 1.2249x over previous
"""Causal single-head attention (B=4, S=2048, D=1024, fp32) on 8 TRN2 NeuronCores.

Sharding: core c <-> (batch c//2, parity c%2). Each core owns the 8 even or
odd 128-row query tiles of its batch (balanced causal work, uniform extents
2j+2 with a 2-tile parity mask). Q is projected locally. The K/V projections
are split across the pair: parity 0 computes K^T for all 2048 keys, parity 1
computes V — the *same* instruction stream on both (stationary [128,128]
blocks x moving [128,1024]), with x and W in swapped roles via the packed
PSTAT/PMOV inputs. Products are exchanged per 1024-key pass as bf16 blobs
through a pairwise DRAM AllGather, then flash-style causal attention runs
over the pass's keys (scores/AV in bf16, accum fp32, partial O carried in
SBUF between passes).

Self-contained: hardcodes shapes; reads nothing from disk.
"""
import sys

import numpy as np

try:
    from concourse import bass, bacc, tile
except ImportError:  # concourse ships with the container, not this file
    for _p in ("/opt/trn_rl_repo", "/root/.axon_site/_ro/trn_rl_repo"):
        if _p not in sys.path:
            sys.path.append(_p)
    from concourse import bass, bacc, tile
from concourse import mybir
from concourse.bass_utils import run_bass_kernel_spmd

dt = mybir.dt
AF = mybir.ActivationFunctionType

B, S, D = 4, 2048, 1024
P = 128
ND = D // P          # 8 d-tiles (contraction of projections)
NO = 8               # out tiles per projection pass (e-tiles or s-tiles)
SLOTS = 8            # q-tiles per core
HT = 8               # k-tiles per pass
NCORES = 8
SCALE = 1.0 / float(np.sqrt(D))
NEG = -1.0e30
GROUPS = [[0, 1], [2, 3], [4, 5], [6, 7]]
SIM_LOCAL_CC = False  # replace collectives with local DMAs (TimelineSim only)
NO_CC = False         # timing probe: skip exchange, read own blob (wrong data)
# slot j of parity r holds query tile 2j+r; uniform program extent 2j+2 k-tiles
# with a parity-dependent 2-tile mask finishing the causal boundary


def _emit_body(nc, tc, pools, aps, rep):
    (sb_const, sb_xq, sb_qt, sb_kt, sb_vt, sb_wblk, sb_wmov, sb_stage,
     sb_p, sb_pt, sb_sums, sb_obuf, sb_ojoin, dram, ps) = pools
    xqT, Wq, PSTAT, PMOV, mask, ident, O = aps

    ident_b = sb_const.tile([P, P], dt.bfloat16, tag="ident", name="ident_b")
    nc.sync.dma_start(ident_b[:], ident[:])
    mask_sb = sb_const.tile([P, 2 * P], dt.float32, tag="mask", name="mask_sb")
    nc.sync.dma_start(mask_sb[:], mask[:])

    # ---------- K-or-V projection per pass + pairwise exchange ----------
    bounce_in = [dram.tile([P, NO * D], dt.bfloat16, tag=f"bin{p}",
                           name=f"bin{p}_{rep}") for p in range(2)]
    bounce_out = [dram.tile([2 * P, NO * D], dt.bfloat16, tag=f"bout{p}",
                            name=f"bout{p}_{rep}") for p in range(2)]
    for p in range(2):
        wmov = sb_wmov.tile([P, ND * D], dt.bfloat16, tag="wmov", name=f"wmov{p}")
        for q in range(4):
            nc.sync.dma_start(wmov[:, q * 2048:(q + 1) * 2048],
                              PMOV[0:P, p * 8192 + q * 2048:p * 8192 + (q + 1) * 2048])
        for o in range(NO):
            wst = sb_wblk.tile([P, ND * P], dt.bfloat16, tag="kvstat", name="wst")
            nc.sync.dma_start(wst[:], PSTAT[0:P, p * 8192 + o * 1024:p * 8192 + (o + 1) * 1024])
            pr0 = ps.tile([P, 512], dt.float32, tag="proj", name="pr0")
            pr1 = ps.tile([P, 512], dt.float32, tag="proj", name="pr1")
            for d in range(ND):
                nc.tensor.matmul(pr0[:], wst[:, d * P:(d + 1) * P],
                                 wmov[:, d * D:d * D + 512],
                                 start=(d == 0), stop=(d == ND - 1))
                nc.tensor.matmul(pr1[:], wst[:, d * P:(d + 1) * P],
                                 wmov[:, d * D + 512:(d + 1) * D],
                                 start=(d == 0), stop=(d == ND - 1))
            stg = sb_stage.tile([P, D], dt.bfloat16, tag="stage", name="stg")
            nc.vector.tensor_copy(stg[:, 0:512], pr0[:])
            nc.vector.tensor_copy(stg[:, 512:1024], pr1[:])
            nc.sync.dma_start(bounce_in[p][:, o * D:(o + 1) * D], stg[:])
        if NO_CC:
            pass
        elif SIM_LOCAL_CC:
            nc.gpsimd.dma_start(bounce_out[p][0:P, :], bounce_in[p][:])
            nc.gpsimd.dma_start(bounce_out[p][P:2 * P, :], bounce_in[p][:])
        else:
            nc.gpsimd.collective_compute(
                "AllGather", mybir.AluOpType.bypass, replica_groups=GROUPS,
                ins=[bounce_in[p][:].opt()], outs=[bounce_out[p][:].opt()])

    # ---------- Q^T projection (own queries), bf16 out ----------
    xq = [sb_xq.tile([P, SLOTS * P], dt.float32r, tag=f"xq{d}", name=f"xq{d}")
          for d in range(ND)]
    for d in range(ND):
        nc.sync.dma_start(xq[d][:], xqT[d * P:(d + 1) * P, :])
    qt = [sb_qt.tile([P, SLOTS * P], dt.bfloat16, tag=f"qt{e}", name=f"qt{e}")
          for e in range(NO)]
    for e in range(NO):
        wb = sb_wblk.tile([P, ND * P], dt.float32r, tag="wstat", name="wb")
        nc.sync.dma_start(wb[:], Wq[e * P:(e + 1) * P, :])
        qp0 = ps.tile([P, 512], dt.float32, tag="proj", name="qp0")
        qp1 = ps.tile([P, 512], dt.float32, tag="proj", name="qp1")
        for d in range(ND):
            nc.tensor.matmul(qp0[:], wb[:, d * P:(d + 1) * P], xq[d][:, 0:512],
                             start=(d == 0), stop=(d == ND - 1))
            nc.tensor.matmul(qp1[:], wb[:, d * P:(d + 1) * P], xq[d][:, 512:1024],
                             start=(d == 0), stop=(d == ND - 1))
        nc.vector.tensor_copy(qt[e][:, 0:512], qp0[:])
        nc.vector.tensor_copy(qt[e][:, 512:1024], qp1[:])

    # per-slot softmax chunk sums + norm scratch (cols 4j..4j+3 sums; 32+2j ssum; 33+2j rec)
    sums = sb_sums.tile([P, 48], dt.float32, tag="sums", name="sums")
    obuf = [sb_obuf.tile([P, D], dt.float32, tag=f"ob{j}", name=f"ob{j}")
            for j in range(4)]          # carry for slots 4..7
    chunk_ctr = [0] * SLOTS

    for p in range(2):
        # kt/vt for this pass from the exchanged blobs
        kt = [sb_kt.tile([P, HT * P], dt.bfloat16, tag=f"kt{e}", name=f"kt{e}")
              for e in range(NO)]
        vt = [sb_vt.tile([P, D], dt.bfloat16, tag=f"vt{t}", name=f"vt{t}")
              for t in range(HT)]
        ksrc = bounce_in[p] if NO_CC else bounce_out[p][0:P, :]
        vsrc = bounce_in[p] if NO_CC else bounce_out[p][P:2 * P, :]
        for e in range(NO):
            nc.sync.dma_start(kt[e][:], ksrc[:, e * D:(e + 1) * D])
        for t in range(HT):
            nc.sync.dma_start(vt[t][:], vsrc[:, t * D:(t + 1) * D])

        for j in range(SLOTS):
            ext = 2 * j + 2                     # uniform extent in k-tiles
            t0 = HT * p
            t1 = min(ext, HT * (p + 1))
            if t1 <= t0:
                continue
            final_pass = (ext - 1) // HT == p

            o_ps = [ps.tile([P, 512], dt.float32, tag="o", name=f"ops{ec}")
                    for ec in range(2)]

            c = t0 * P
            while c < t1 * P:
                cw = 512 if t1 * P - c >= 512 else t1 * P - c
                sps = ps.tile([P, cw], dt.float32, tag="s", name="sps")
                for e in range(NO):
                    nc.tensor.matmul(sps[:], qt[e][:, j * P:(j + 1) * P],
                                     kt[e][:, c - p * 1024: c - p * 1024 + cw],
                                     start=(e == 0), stop=(e == NO - 1))
                if final_pass and c + cw == ext * P:
                    off = cw - 2 * P
                    nc.vector.tensor_add(sps[:, off:off + 2 * P],
                                         sps[:, off:off + 2 * P], mask_sb[:])
                pch = sb_p.tile([P, cw], dt.bfloat16, tag="p", name="pch")
                ci = chunk_ctr[j]
                chunk_ctr[j] += 1
                nc.scalar.activation(pch[:], sps[:], AF.Exp, scale=SCALE,
                                     accum_out=sums[:, 4 * j + ci:4 * j + ci + 1])
                for ti in range(cw // P):
                    t = c // P + ti
                    ptb = sb_pt.tile([P, P], dt.bfloat16, tag="pt", name="ptb")
                    nc.sync.dma_start(ptb[:], pch[:, ti * P:(ti + 1) * P], transpose=True)
                    for ec in range(2):
                        nc.tensor.matmul(o_ps[ec][:], ptb[:],
                                         vt[t - HT * p][:, ec * 512:(ec + 1) * 512],
                                         start=(t == t0), stop=(t == t1 - 1))
                c += cw

            if final_pass:
                ssum = sums[:, 32 + 2 * j:33 + 2 * j]
                nc.vector.reduce_sum(ssum, sums[:, 4 * j:4 * j + chunk_ctr[j]],
                                     axis=mybir.AxisListType.X)
                rec = sums[:, 33 + 2 * j:34 + 2 * j]
                nc.vector.reciprocal(rec, ssum)
                ob = sb_stage.tile([P, D], dt.float32, tag="obf", name="obfin")
                for ec in range(2):
                    if ext > HT:
                        oj = sb_ojoin.tile([P, 512], dt.float32, tag="ojoin", name="oj")
                        nc.vector.tensor_add(oj[:], o_ps[ec][:],
                                             obuf[j - 4][:, ec * 512:(ec + 1) * 512])
                        nc.scalar.mul(ob[:, ec * 512:(ec + 1) * 512], oj[:], rec)
                    else:
                        nc.scalar.mul(ob[:, ec * 512:(ec + 1) * 512], o_ps[ec][:], rec)
                nc.sync.dma_start(O[j * P:(j + 1) * P, :], ob[:])
            else:
                for ec in range(2):
                    nc.vector.tensor_copy(obuf[j - 4][:, ec * 512:(ec + 1) * 512],
                                          o_ps[ec][:])


def build_program(reps: int = 1):
    nc = bacc.Bacc("TRN2", target_bir_lowering=False, debug=False, num_devices=NCORES)

    xqT_t = nc.dram_tensor("xqT", [D, SLOTS * P], dt.float32r, kind="ExternalInput")
    Wq_t = nc.dram_tensor("Wq", [D, D], dt.float32r, kind="ExternalInput")
    PSTAT_t = nc.dram_tensor("PSTAT", [P, 2 * NO * ND * P], dt.bfloat16, kind="ExternalInput")
    PMOV_t = nc.dram_tensor("PMOV", [P, 2 * ND * D], dt.bfloat16, kind="ExternalInput")
    mask_t = nc.dram_tensor("mask", [P, 2 * P], dt.float32, kind="ExternalInput")
    ident_t = nc.dram_tensor("ident", [P, P], dt.bfloat16, kind="ExternalInput")
    O_t = nc.dram_tensor("O", [SLOTS * P, D], dt.float32, kind="ExternalOutput")

    with tile.TileContext(nc) as tc:
        with (
            tc.tile_pool(name="const", bufs=1) as sb_const,
            tc.tile_pool(name="xq", bufs=1) as sb_xq,
            tc.tile_pool(name="qt", bufs=1) as sb_qt,
            tc.tile_pool(name="kt", bufs=2) as sb_kt,
            tc.tile_pool(name="vt", bufs=2) as sb_vt,
            tc.tile_pool(name="wstat", bufs=3) as sb_wblk,
            tc.tile_pool(name="wmov", bufs=2) as sb_wmov,
            tc.tile_pool(name="stage", bufs=3) as sb_stage,
            tc.tile_pool(name="p", bufs=3) as sb_p,
            tc.tile_pool(name="pt", bufs=6) as sb_pt,
            tc.tile_pool(name="sums", bufs=1) as sb_sums,
            tc.tile_pool(name="obuf", bufs=1) as sb_obuf,
            tc.tile_pool(name="ojoin", bufs=2) as sb_ojoin,
            tc.tile_pool(name="dram", bufs=2, space=bass.MemorySpace.DRAM) as dram,
            tc.tile_pool(name="ps", bufs=2, space=bass.MemorySpace.PSUM) as ps,
        ):
            pools = (sb_const, sb_xq, sb_qt, sb_kt, sb_vt, sb_wblk, sb_wmov,
                     sb_stage, sb_p, sb_pt, sb_sums, sb_obuf, sb_ojoin, dram, ps)
            # reps are unrolled (python loop, NOT tc.For_i): collectives are
            # not supported inside hardware loops on this runtime path.
            for r in range(reps):
                aps = (xqT_t.ap(), Wq_t.ap(), PSTAT_t.ap(), PMOV_t.ap(),
                       mask_t.ap(), ident_t.ap(), O_t.ap())
                _emit_body(nc, tc, pools, aps, r)

    nc.compile()
    return nc


def round_f32r(a):
    """Round fp32 to the fp32r grid (low 12 mantissa bits dropped, nearest-even)."""
    u = np.ascontiguousarray(a, np.float32).view(np.uint32).copy()
    low = u & np.uint32(0xFFF)
    base = u & np.uint32(0xFFFFF000)
    up = (low > 0x800) | ((low == 0x800) & (((base >> 12) & 1) == 1))
    base[up] += np.uint32(0x1000)
    return base.view(np.float32)


def pack_w_blocks(W):
    """[D, D] -> [D, D]: row (e*P+p), col (d*P+c) = W[d*P+p, e*P+c]."""
    return np.ascontiguousarray(
        W.reshape(ND, P, NO, P).transpose(2, 1, 0, 3).reshape(D, D))


def make_in_maps(x, Wq, Wk, Wv):
    import ml_dtypes
    x = round_f32r(x.reshape(B, S, D))
    Wq_p = pack_w_blocks(round_f32r(Wq))
    Wk_p = pack_w_blocks(round_f32r(Wk))
    Wv_r = round_f32r(Wv)
    ident = np.eye(P, dtype=np.float32).astype(ml_dtypes.bfloat16)
    tri = np.where(np.arange(P)[None, :] <= np.arange(P)[:, None], 0.0, NEG).astype(np.float32)
    masks = [
        np.concatenate([tri, np.full((P, P), NEG, np.float32)], axis=1),   # parity 0
        np.concatenate([np.zeros((P, P), np.float32), tri], axis=1),       # parity 1
    ]
    xT = [np.ascontiguousarray(x[b].T) for b in range(B)]   # [D, S] per batch

    # K-core PSTAT: pack[o*P+p, d*P+c] -> [p, o*1024 + d*128 + c], same both passes
    pk = Wk_p.reshape(NO, P, ND * P).transpose(1, 0, 2).reshape(P, NO * ND * P)
    PSTAT_K = np.ascontiguousarray(np.concatenate([pk, pk], axis=1)).astype(ml_dtypes.bfloat16)
    # V-core PMOV: Wv[d*P+p, c] -> [p, d*1024 + c], same both passes
    pv = Wv_r.reshape(ND, P, D).transpose(1, 0, 2).reshape(P, ND * D)
    PMOV_V = np.ascontiguousarray(np.concatenate([pv, pv], axis=1)).astype(ml_dtypes.bfloat16)

    in_maps = []
    for c in range(NCORES):
        b, r = c // 2, c % 2
        xTb = xT[b]
        cols = np.concatenate([np.arange((2 * j + r) * P, (2 * j + r + 1) * P)
                               for j in range(SLOTS)])
        xqTb = np.ascontiguousarray(xTb[:, cols])
        if r == 0:
            pstat = PSTAT_K
            # PMOV: xT[d*P+p, pass*1024 + c] -> [p, pass*8192 + d*1024 + c]
            pm = xTb.reshape(ND, P, 2, D).transpose(1, 2, 0, 3)
            pmov = np.ascontiguousarray(pm.reshape(P, 2 * ND * D)).astype(ml_dtypes.bfloat16)
        else:
            # PSTAT: xT[d*P+p, pass*1024 + o*128 + c] -> [p, pass*8192 + o*1024 + d*128 + c]
            t4 = xTb.reshape(ND, P, 2, NO, P).transpose(1, 2, 3, 0, 4)
            pstat = np.ascontiguousarray(t4.reshape(P, 2 * NO * ND * P)).astype(ml_dtypes.bfloat16)
            pmov = PMOV_V
        in_maps.append({
            "xqT": xqTb, "Wq": Wq_p, "PSTAT": pstat, "PMOV": pmov,
            "mask": masks[r], "ident": ident,
        })
    return in_maps


def assemble_output(results):
    out = np.empty((B, S, D), dtype=np.float32)
    for c in range(NCORES):
        b, r = c // 2, c % 2
        oc = results[c]["O"].reshape(SLOTS, P, D)
        for j in range(SLOTS):
            out[b, (2 * j + r) * P:(2 * j + r + 1) * P, :] = oc[j]
    return out


_nc_cache = {}


def _get_program(reps: int = 1):
    if reps not in _nc_cache:
        _nc_cache[reps] = build_program(reps)
    return _nc_cache[reps]


def kernel(x, Wq, Wk, Wv):
    x = np.asarray(x, dtype=np.float32)
    Wq = np.asarray(Wq, dtype=np.float32)
    Wk = np.asarray(Wk, dtype=np.float32)
    Wv = np.asarray(Wv, dtype=np.float32)
    nc = _get_program(1)
    in_maps = make_in_maps(x, Wq, Wk, Wv)
    results = run_bass_kernel_spmd(nc, in_maps, list(range(NCORES))).results
    return assemble_output(results)


# revision 25
# speedup vs baseline: 1.6583x; 1.3538x over previous
"""Causal single-head attention (B=4, S=2048, D=1024, fp32) on 8 TRN2 NeuronCores.

Sharding: core c <-> (batch c//2, parity c%2). Each core owns the 8 even or
odd 128-row query tiles of its batch (balanced causal work, uniform extents
2j+2 with a 2-tile parity mask). Q is projected locally. The K/V projections
are split across the pair: parity 0 computes K^T for all 2048 keys, parity 1
computes V — the *same* instruction stream on both (stationary [128,128]
blocks x moving [128,1024]), with x and W in swapped roles via the packed
PSTAT/PMOV inputs. Products are exchanged per 1024-key pass as bf16 blobs
through a pairwise DRAM AllGather, then flash-style causal attention runs
over the pass's keys (scores/AV in bf16, accum fp32, partial O carried in
SBUF between passes).

Self-contained: hardcodes shapes; reads nothing from disk.
"""
import sys

import numpy as np

try:
    from concourse import bass, bacc, tile
except ImportError:  # concourse ships with the container, not this file
    for _p in ("/opt/trn_rl_repo", "/root/.axon_site/_ro/trn_rl_repo"):
        if _p not in sys.path:
            sys.path.append(_p)
    from concourse import bass, bacc, tile
from concourse import mybir
from concourse.bass_utils import run_bass_kernel_spmd

dt = mybir.dt
AF = mybir.ActivationFunctionType

B, S, D = 4, 2048, 1024
P = 128
ND = D // P          # 8 d-tiles (contraction of projections)
NO = 8               # out tiles per projection pass (e-tiles or s-tiles)
SLOTS = 8            # q-tiles per core
HT = 8               # k-tiles per pass
NCORES = 8
SCALE = 1.0 / float(np.sqrt(D))
NEG = -1.0e30
GROUPS = [[0, 1], [2, 3], [4, 5], [6, 7]]
SIM_LOCAL_CC = False  # replace collectives with local DMAs (TimelineSim only)
NO_CC = False         # timing probe: skip exchange, read own blob (wrong data)
# slot j of parity r holds query tile 2j+r; uniform program extent 2j+2 k-tiles
# with a parity-dependent 2-tile mask finishing the causal boundary


def _emit_body(nc, tc, pools, aps, rep):
    (sb_const, sb_xq, sb_qt, sb_kt, sb_vt, sb_wblk, sb_wmov, sb_stage,
     sb_p, sb_pt, sb_sums, sb_obuf, sb_ojoin, dram, ps) = pools
    xqT, Wq, PSTAT, PMOV, mask, ident, O = aps

    ident_b = sb_const.tile([P, P], dt.bfloat16, tag="ident", name="ident_b")
    nc.sync.dma_start(ident_b[:], ident[:])
    mask_sb = sb_const.tile([P, 2 * P], dt.float32, tag="mask", name="mask_sb")
    nc.sync.dma_start(mask_sb[:], mask[:])

    # ---------- K-or-V projection per pass + pairwise exchange ----------
    bounce_in = [dram.tile([P, NO * D], dt.bfloat16, tag=f"bin{p}",
                           name=f"bin{p}_{rep}") for p in range(2)]
    bounce_out = [dram.tile([2 * P, NO * D], dt.bfloat16, tag=f"bout{p}",
                            name=f"bout{p}_{rep}") for p in range(2)]
    for p in range(2):
        wmov = sb_wmov.tile([P, ND * D], dt.bfloat16, tag="wmov", name=f"wmov{p}")
        for q in range(4):
            nc.sync.dma_start(wmov[:, q * 2048:(q + 1) * 2048],
                              PMOV[0:P, p * 8192 + q * 2048:p * 8192 + (q + 1) * 2048])
        for o in range(NO):
            wst = sb_wblk.tile([P, ND * P], dt.bfloat16, tag="kvstat", name="wst")
            nc.sync.dma_start(wst[:], PSTAT[0:P, p * 8192 + o * 1024:p * 8192 + (o + 1) * 1024])
            pr0 = ps.tile([P, 512], dt.float32, tag="proj", name="pr0")
            pr1 = ps.tile([P, 512], dt.float32, tag="proj", name="pr1")
            for d in range(ND):
                nc.tensor.matmul(pr0[:], wst[:, d * P:(d + 1) * P],
                                 wmov[:, d * D:d * D + 512],
                                 start=(d == 0), stop=(d == ND - 1))
                nc.tensor.matmul(pr1[:], wst[:, d * P:(d + 1) * P],
                                 wmov[:, d * D + 512:(d + 1) * D],
                                 start=(d == 0), stop=(d == ND - 1))
            stg = sb_stage.tile([P, D], dt.bfloat16, tag="stage", name="stg")
            nc.vector.tensor_copy(stg[:, 0:512], pr0[:])
            nc.vector.tensor_copy(stg[:, 512:1024], pr1[:])
            nc.sync.dma_start(bounce_in[p][:, o * D:(o + 1) * D], stg[:])
        if NO_CC:
            pass
        elif SIM_LOCAL_CC:
            nc.gpsimd.dma_start(bounce_out[p][0:P, :], bounce_in[p][:])
            nc.gpsimd.dma_start(bounce_out[p][P:2 * P, :], bounce_in[p][:])
        else:
            nc.gpsimd.collective_compute(
                "AllGather", mybir.AluOpType.bypass, replica_groups=GROUPS,
                ins=[bounce_in[p][:].opt()], outs=[bounce_out[p][:].opt()])

    # ---------- Q^T projection (own queries), bf16 out ----------
    xq = [sb_xq.tile([P, SLOTS * P], dt.float32r, tag=f"xq{d}", name=f"xq{d}")
          for d in range(ND)]
    for d in range(ND):
        nc.sync.dma_start(xq[d][:], xqT[d * P:(d + 1) * P, :])
    qt = [sb_qt.tile([P, SLOTS * P], dt.bfloat16, tag=f"qt{e}", name=f"qt{e}")
          for e in range(NO)]
    for e in range(NO):
        wb = sb_wblk.tile([P, ND * P], dt.float32r, tag="wstat", name="wb")
        nc.sync.dma_start(wb[:], Wq[e * P:(e + 1) * P, :])
        qp0 = ps.tile([P, 512], dt.float32, tag="proj", name="qp0")
        qp1 = ps.tile([P, 512], dt.float32, tag="proj", name="qp1")
        for d in range(ND):
            nc.tensor.matmul(qp0[:], wb[:, d * P:(d + 1) * P], xq[d][:, 0:512],
                             start=(d == 0), stop=(d == ND - 1))
            nc.tensor.matmul(qp1[:], wb[:, d * P:(d + 1) * P], xq[d][:, 512:1024],
                             start=(d == 0), stop=(d == ND - 1))
        nc.vector.tensor_copy(qt[e][:, 0:512], qp0[:])
        nc.vector.tensor_copy(qt[e][:, 512:1024], qp1[:])

    # per-slot softmax chunk sums + norm scratch (cols 4j..4j+3 sums; 32+2j ssum; 33+2j rec)
    sums = sb_sums.tile([P, 48], dt.float32, tag="sums", name="sums")
    obuf = [sb_obuf.tile([P, D], dt.float32, tag=f"ob{j}", name=f"ob{j}")
            for j in range(4)]          # carry for slots 4..7
    chunk_ctr = [0] * SLOTS

    for p in range(2):
        # kt/vt for this pass from the exchanged blobs
        kt = [sb_kt.tile([P, HT * P], dt.bfloat16, tag=f"kt{e}", name=f"kt{e}")
              for e in range(NO)]
        vt = [sb_vt.tile([P, D], dt.bfloat16, tag=f"vt{t}", name=f"vt{t}")
              for t in range(HT)]
        ksrc = bounce_in[p] if NO_CC else bounce_out[p][0:P, :]
        vsrc = bounce_in[p] if NO_CC else bounce_out[p][P:2 * P, :]
        for e in range(NO):
            nc.sync.dma_start(kt[e][:], ksrc[:, e * D:(e + 1) * D])
        for t in range(HT):
            nc.sync.dma_start(vt[t][:], vsrc[:, t * D:(t + 1) * D])

        for j in range(SLOTS):
            ext = 2 * j + 2                     # uniform extent in k-tiles
            t0 = HT * p
            t1 = min(ext, HT * (p + 1))
            if t1 <= t0:
                continue
            final_pass = (ext - 1) // HT == p

            o_ps = [ps.tile([P, 512], dt.float32, tag="o", name=f"ops{ec}")
                    for ec in range(2)]

            c = t0 * P
            while c < t1 * P:
                cw = 512 if t1 * P - c >= 512 else t1 * P - c
                sps = ps.tile([P, cw], dt.float32, tag="s", name="sps")
                for e in range(NO):
                    nc.tensor.matmul(sps[:], qt[e][:, j * P:(j + 1) * P],
                                     kt[e][:, c - p * 1024: c - p * 1024 + cw],
                                     start=(e == 0), stop=(e == NO - 1))
                if final_pass and c + cw == ext * P:
                    off = cw - 2 * P
                    nc.vector.tensor_add(sps[:, off:off + 2 * P],
                                         sps[:, off:off + 2 * P], mask_sb[:])
                pch = sb_p.tile([P, cw], dt.bfloat16, tag="p", name="pch")
                ci = chunk_ctr[j]
                chunk_ctr[j] += 1
                nc.scalar.activation(pch[:], sps[:], AF.Exp, scale=SCALE,
                                     accum_out=sums[:, 4 * j + ci:4 * j + ci + 1])
                for ti in range(cw // P):
                    t = c // P + ti
                    ptp = ps.tile([P, P], dt.bfloat16, tag="tr", name="ptp")
                    nc.tensor.transpose(ptp[:], pch[:, ti * P:(ti + 1) * P], ident_b[:])
                    ptb = sb_pt.tile([P, P], dt.bfloat16, tag="pt", name="ptb")
                    nc.vector.tensor_copy(ptb[:], ptp[:])
                    for ec in range(2):
                        nc.tensor.matmul(o_ps[ec][:], ptb[:],
                                         vt[t - HT * p][:, ec * 512:(ec + 1) * 512],
                                         start=(t == t0), stop=(t == t1 - 1))
                c += cw

            if final_pass:
                ssum = sums[:, 32 + 2 * j:33 + 2 * j]
                nc.vector.reduce_sum(ssum, sums[:, 4 * j:4 * j + chunk_ctr[j]],
                                     axis=mybir.AxisListType.X)
                rec = sums[:, 33 + 2 * j:34 + 2 * j]
                nc.vector.reciprocal(rec, ssum)
                ob = sb_stage.tile([P, D], dt.float32, tag="obf", name="obfin")
                for ec in range(2):
                    if ext > HT:
                        oj = sb_ojoin.tile([P, 512], dt.float32, tag="ojoin", name="oj")
                        nc.vector.tensor_add(oj[:], o_ps[ec][:],
                                             obuf[j - 4][:, ec * 512:(ec + 1) * 512])
                        nc.scalar.mul(ob[:, ec * 512:(ec + 1) * 512], oj[:], rec)
                    else:
                        nc.scalar.mul(ob[:, ec * 512:(ec + 1) * 512], o_ps[ec][:], rec)
                nc.sync.dma_start(O[j * P:(j + 1) * P, :], ob[:])
            else:
                for ec in range(2):
                    nc.vector.tensor_copy(obuf[j - 4][:, ec * 512:(ec + 1) * 512],
                                          o_ps[ec][:])


def build_program(reps: int = 1):
    nc = bacc.Bacc("TRN2", target_bir_lowering=False, debug=False, num_devices=NCORES)

    xqT_t = nc.dram_tensor("xqT", [D, SLOTS * P], dt.float32r, kind="ExternalInput")
    Wq_t = nc.dram_tensor("Wq", [D, D], dt.float32r, kind="ExternalInput")
    PSTAT_t = nc.dram_tensor("PSTAT", [P, 2 * NO * ND * P], dt.bfloat16, kind="ExternalInput")
    PMOV_t = nc.dram_tensor("PMOV", [P, 2 * ND * D], dt.bfloat16, kind="ExternalInput")
    mask_t = nc.dram_tensor("mask", [P, 2 * P], dt.float32, kind="ExternalInput")
    ident_t = nc.dram_tensor("ident", [P, P], dt.bfloat16, kind="ExternalInput")
    O_t = nc.dram_tensor("O", [SLOTS * P, D], dt.float32, kind="ExternalOutput")

    with tile.TileContext(nc) as tc:
        with (
            tc.tile_pool(name="const", bufs=1) as sb_const,
            tc.tile_pool(name="xq", bufs=1) as sb_xq,
            tc.tile_pool(name="qt", bufs=1) as sb_qt,
            tc.tile_pool(name="kt", bufs=2) as sb_kt,
            tc.tile_pool(name="vt", bufs=2) as sb_vt,
            tc.tile_pool(name="wstat", bufs=3) as sb_wblk,
            tc.tile_pool(name="wmov", bufs=2) as sb_wmov,
            tc.tile_pool(name="stage", bufs=3) as sb_stage,
            tc.tile_pool(name="p", bufs=3) as sb_p,
            tc.tile_pool(name="pt", bufs=6) as sb_pt,
            tc.tile_pool(name="sums", bufs=1) as sb_sums,
            tc.tile_pool(name="obuf", bufs=1) as sb_obuf,
            tc.tile_pool(name="ojoin", bufs=2) as sb_ojoin,
            tc.tile_pool(name="dram", bufs=2, space=bass.MemorySpace.DRAM) as dram,
            tc.tile_pool(name="ps", bufs=2, space=bass.MemorySpace.PSUM) as ps,
        ):
            pools = (sb_const, sb_xq, sb_qt, sb_kt, sb_vt, sb_wblk, sb_wmov,
                     sb_stage, sb_p, sb_pt, sb_sums, sb_obuf, sb_ojoin, dram, ps)
            # reps are unrolled (python loop, NOT tc.For_i): collectives are
            # not supported inside hardware loops on this runtime path.
            for r in range(reps):
                aps = (xqT_t.ap(), Wq_t.ap(), PSTAT_t.ap(), PMOV_t.ap(),
                       mask_t.ap(), ident_t.ap(), O_t.ap())
                _emit_body(nc, tc, pools, aps, r)

    nc.compile()
    return nc


def round_f32r(a):
    """Round fp32 to the fp32r grid (low 12 mantissa bits dropped, nearest-even)."""
    u = np.ascontiguousarray(a, np.float32).view(np.uint32).copy()
    low = u & np.uint32(0xFFF)
    base = u & np.uint32(0xFFFFF000)
    up = (low > 0x800) | ((low == 0x800) & (((base >> 12) & 1) == 1))
    base[up] += np.uint32(0x1000)
    return base.view(np.float32)


def pack_w_blocks(W):
    """[D, D] -> [D, D]: row (e*P+p), col (d*P+c) = W[d*P+p, e*P+c]."""
    return np.ascontiguousarray(
        W.reshape(ND, P, NO, P).transpose(2, 1, 0, 3).reshape(D, D))


def make_in_maps(x, Wq, Wk, Wv):
    import ml_dtypes
    x = round_f32r(x.reshape(B, S, D))
    Wq_p = pack_w_blocks(round_f32r(Wq))
    Wk_p = pack_w_blocks(round_f32r(Wk))
    Wv_r = round_f32r(Wv)
    ident = np.eye(P, dtype=np.float32).astype(ml_dtypes.bfloat16)
    tri = np.where(np.arange(P)[None, :] <= np.arange(P)[:, None], 0.0, NEG).astype(np.float32)
    masks = [
        np.concatenate([tri, np.full((P, P), NEG, np.float32)], axis=1),   # parity 0
        np.concatenate([np.zeros((P, P), np.float32), tri], axis=1),       # parity 1
    ]
    xT = [np.ascontiguousarray(x[b].T) for b in range(B)]   # [D, S] per batch

    # K-core PSTAT: pack[o*P+p, d*P+c] -> [p, o*1024 + d*128 + c], same both passes
    pk = Wk_p.reshape(NO, P, ND * P).transpose(1, 0, 2).reshape(P, NO * ND * P)
    PSTAT_K = np.ascontiguousarray(np.concatenate([pk, pk], axis=1)).astype(ml_dtypes.bfloat16)
    # V-core PMOV: Wv[d*P+p, c] -> [p, d*1024 + c], same both passes
    pv = Wv_r.reshape(ND, P, D).transpose(1, 0, 2).reshape(P, ND * D)
    PMOV_V = np.ascontiguousarray(np.concatenate([pv, pv], axis=1)).astype(ml_dtypes.bfloat16)

    in_maps = []
    for c in range(NCORES):
        b, r = c // 2, c % 2
        xTb = xT[b]
        cols = np.concatenate([np.arange((2 * j + r) * P, (2 * j + r + 1) * P)
                               for j in range(SLOTS)])
        xqTb = np.ascontiguousarray(xTb[:, cols])
        if r == 0:
            pstat = PSTAT_K
            # PMOV: xT[d*P+p, pass*1024 + c] -> [p, pass*8192 + d*1024 + c]
            pm = xTb.reshape(ND, P, 2, D).transpose(1, 2, 0, 3)
            pmov = np.ascontiguousarray(pm.reshape(P, 2 * ND * D)).astype(ml_dtypes.bfloat16)
        else:
            # PSTAT: xT[d*P+p, pass*1024 + o*128 + c] -> [p, pass*8192 + o*1024 + d*128 + c]
            t4 = xTb.reshape(ND, P, 2, NO, P).transpose(1, 2, 3, 0, 4)
            pstat = np.ascontiguousarray(t4.reshape(P, 2 * NO * ND * P)).astype(ml_dtypes.bfloat16)
            pmov = PMOV_V
        in_maps.append({
            "xqT": xqTb, "Wq": Wq_p, "PSTAT": pstat, "PMOV": pmov,
            "mask": masks[r], "ident": ident,
        })
    return in_maps


def assemble_output(results):
    out = np.empty((B, S, D), dtype=np.float32)
    for c in range(NCORES):
        b, r = c // 2, c % 2
        oc = results[c]["O"].reshape(SLOTS, P, D)
        for j in range(SLOTS):
            out[b, (2 * j + r) * P:(2 * j + r + 1) * P, :] = oc[j]
    return out


_nc_cache = {}


def _get_program(reps: int = 1):
    if reps not in _nc_cache:
        _nc_cache[reps] = build_program(reps)
    return _nc_cache[reps]


def kernel(x, Wq, Wk, Wv):
    x = np.asarray(x, dtype=np.float32)
    Wq = np.asarray(Wq, dtype=np.float32)
    Wk = np.asarray(Wk, dtype=np.float32)
    Wv = np.asarray(Wv, dtype=np.float32)
    nc = _get_program(1)
    in_maps = make_in_maps(x, Wq, Wk, Wv)
    results = run_bass_kernel_spmd(nc, in_maps, list(range(NCORES))).results
    return assemble_output(results)


# revision 27
# speedup vs baseline: 1.7874x; 1.0779x over previous
"""Causal single-head attention (B=4, S=2048, D=1024, fp32) on 8 TRN2 NeuronCores.

Sharding: core c <-> (batch c//2, parity c%2). Each core owns the 8 even or
odd 128-row query tiles of its batch (balanced causal work, uniform extents
2j+2 with a 2-tile parity mask). Q is projected locally. The K/V projections
are split across the pair: parity 0 computes K^T for all 2048 keys, parity 1
computes V — the *same* instruction stream on both (stationary [128,128]
blocks x moving [128,1024]), with x and W in swapped roles via the packed
PSTAT/PMOV inputs. Products are exchanged per 1024-key pass as bf16 blobs
through a pairwise DRAM AllGather, then flash-style causal attention runs
over the pass's keys (scores/AV in bf16, accum fp32, partial O carried in
SBUF between passes).

Self-contained: hardcodes shapes; reads nothing from disk.
"""
import sys

import numpy as np

try:
    from concourse import bass, bacc, tile
except ImportError:  # concourse ships with the container, not this file
    for _p in ("/opt/trn_rl_repo", "/root/.axon_site/_ro/trn_rl_repo"):
        if _p not in sys.path:
            sys.path.append(_p)
    from concourse import bass, bacc, tile
from concourse import mybir
from concourse.bass_utils import run_bass_kernel_spmd

dt = mybir.dt
AF = mybir.ActivationFunctionType

B, S, D = 4, 2048, 1024
P = 128
ND = D // P          # 8 d-tiles (contraction of projections)
NO = 8               # out tiles per projection pass (e-tiles or s-tiles)
SLOTS = 8            # q-tiles per core
HT = 8               # k-tiles per pass
NCORES = 8
SCALE = 1.0 / float(np.sqrt(D))
NEG = -1.0e30
GROUPS = [[0, 1], [2, 3], [4, 5], [6, 7]]
SIM_LOCAL_CC = False  # replace collectives with local DMAs (TimelineSim only)
NO_CC = False         # timing probe: skip exchange, read own blob (wrong data)
# slot j of parity r holds query tile 2j+r; uniform program extent 2j+2 k-tiles
# with a parity-dependent 2-tile mask finishing the causal boundary


def _emit_body(nc, tc, pools, aps, rep):
    (sb_const, sb_xq, sb_qt, sb_kt, sb_vt, sb_wblk, sb_wmov, sb_stage,
     sb_p, sb_pt, sb_sums, sb_obuf, sb_ojoin, dram, ps) = pools
    xqT, Wq, PSTAT, PMOV, mask, ident, O = aps

    ident_b = sb_const.tile([P, P], dt.bfloat16, tag="ident", name="ident_b")
    nc.sync.dma_start(ident_b[:], ident[:])
    mask_sb = sb_const.tile([P, 2 * P], dt.float32, tag="mask", name="mask_sb")
    nc.sync.dma_start(mask_sb[:], mask[:])

    # ---------- K-or-V projection per pass + pairwise exchange ----------
    bounce_in = [dram.tile([P, NO * D], dt.bfloat16, tag=f"bin{p}",
                           name=f"bin{p}_{rep}") for p in range(2)]
    bounce_out = [dram.tile([2 * P, NO * D], dt.bfloat16, tag=f"bout{p}",
                            name=f"bout{p}_{rep}") for p in range(2)]
    # rolling window-3 prefetch of the [128,1024] stationary blocks; both
    # passes' moving operands loaded up front (wmov pool bufs=2)
    def emit_wst(idx):
        pp, oo = divmod(idx, NO)
        w = sb_wblk.tile([P, ND * P], dt.bfloat16, tag="kvstat", name="wst")
        nc.sync.dma_start(w[:], PSTAT[0:P, pp * 8192 + oo * 1024:pp * 8192 + (oo + 1) * 1024])
        return w

    wmovs = []
    for p in range(2):
        wmov = sb_wmov.tile([P, ND * D], dt.bfloat16, tag="wmov", name=f"wmov{p}")
        wmovs.append(wmov)
    for q in range(4):
        nc.sync.dma_start(wmovs[0][:, q * 2048:(q + 1) * 2048],
                          PMOV[0:P, q * 2048:(q + 1) * 2048])
    wsts = {i: emit_wst(i) for i in range(3)}
    for q in range(4):
        nc.sync.dma_start(wmovs[1][:, q * 2048:(q + 1) * 2048],
                          PMOV[0:P, 8192 + q * 2048:8192 + (q + 1) * 2048])

    for p in range(2):
        wmov = wmovs[p]
        for o in range(NO):
            wst = wsts.pop(p * NO + o)
            pr0 = ps.tile([P, 512], dt.float32, tag="proj", name="pr0")
            pr1 = ps.tile([P, 512], dt.float32, tag="proj", name="pr1")
            for d in range(ND):
                nc.tensor.matmul(pr0[:], wst[:, d * P:(d + 1) * P],
                                 wmov[:, d * D:d * D + 512],
                                 start=(d == 0), stop=(d == ND - 1))
                nc.tensor.matmul(pr1[:], wst[:, d * P:(d + 1) * P],
                                 wmov[:, d * D + 512:(d + 1) * D],
                                 start=(d == 0), stop=(d == ND - 1))
            stg = sb_stage.tile([P, D], dt.bfloat16, tag="stage", name="stg")
            nc.vector.tensor_copy(stg[:, 0:512], pr0[:])
            nc.vector.tensor_copy(stg[:, 512:1024], pr1[:])
            nc.sync.dma_start(bounce_in[p][:, o * D:(o + 1) * D], stg[:])
            nxt = p * NO + o + 3
            if nxt < 2 * NO:
                wsts[nxt] = emit_wst(nxt)
        if NO_CC:
            pass
        elif SIM_LOCAL_CC:
            nc.gpsimd.dma_start(bounce_out[p][0:P, :], bounce_in[p][:])
            nc.gpsimd.dma_start(bounce_out[p][P:2 * P, :], bounce_in[p][:])
        else:
            nc.gpsimd.collective_compute(
                "AllGather", mybir.AluOpType.bypass, replica_groups=GROUPS,
                ins=[bounce_in[p][:].opt()], outs=[bounce_out[p][:].opt()])

    # ---------- Q^T projection (own queries), bf16 out ----------
    xq = [sb_xq.tile([P, SLOTS * P], dt.float32r, tag=f"xq{d}", name=f"xq{d}")
          for d in range(ND)]
    for d in range(ND):
        nc.scalar.dma_start(xq[d][:], xqT[d * P:(d + 1) * P, :])
    qt = [sb_qt.tile([P, SLOTS * P], dt.bfloat16, tag=f"qt{e}", name=f"qt{e}")
          for e in range(NO)]
    for e in range(NO):
        wb = sb_wblk.tile([P, ND * P], dt.float32r, tag="wstat", name="wb")
        nc.scalar.dma_start(wb[:], Wq[e * P:(e + 1) * P, :])
        qp0 = ps.tile([P, 512], dt.float32, tag="proj", name="qp0")
        qp1 = ps.tile([P, 512], dt.float32, tag="proj", name="qp1")
        for d in range(ND):
            nc.tensor.matmul(qp0[:], wb[:, d * P:(d + 1) * P], xq[d][:, 0:512],
                             start=(d == 0), stop=(d == ND - 1))
            nc.tensor.matmul(qp1[:], wb[:, d * P:(d + 1) * P], xq[d][:, 512:1024],
                             start=(d == 0), stop=(d == ND - 1))
        nc.vector.tensor_copy(qt[e][:, 0:512], qp0[:])
        nc.vector.tensor_copy(qt[e][:, 512:1024], qp1[:])

    # per-slot softmax chunk sums + norm scratch (cols 4j..4j+3 sums; 32+2j ssum; 33+2j rec)
    sums = sb_sums.tile([P, 48], dt.float32, tag="sums", name="sums")
    obuf = [sb_obuf.tile([P, D], dt.float32, tag=f"ob{j}", name=f"ob{j}")
            for j in range(4)]          # carry for slots 4..7
    chunk_ctr = [0] * SLOTS

    for p in range(2):
        # kt/vt for this pass from the exchanged blobs
        kt = [sb_kt.tile([P, HT * P], dt.bfloat16, tag=f"kt{e}", name=f"kt{e}")
              for e in range(NO)]
        vt = [sb_vt.tile([P, D], dt.bfloat16, tag=f"vt{t}", name=f"vt{t}")
              for t in range(HT)]
        ksrc = bounce_in[p] if NO_CC else bounce_out[p][0:P, :]
        vsrc = bounce_in[p] if NO_CC else bounce_out[p][P:2 * P, :]
        for e in range(NO):
            nc.sync.dma_start(kt[e][:], ksrc[:, e * D:(e + 1) * D])
        for t in range(HT):
            nc.sync.dma_start(vt[t][:], vsrc[:, t * D:(t + 1) * D])

        for j in range(SLOTS):
            ext = 2 * j + 2                     # uniform extent in k-tiles
            t0 = HT * p
            t1 = min(ext, HT * (p + 1))
            if t1 <= t0:
                continue
            final_pass = (ext - 1) // HT == p

            o_ps = [ps.tile([P, 512], dt.float32, tag="o", name=f"ops{ec}")
                    for ec in range(2)]

            c = t0 * P
            while c < t1 * P:
                cw = 512 if t1 * P - c >= 512 else t1 * P - c
                sps = ps.tile([P, cw], dt.float32, tag="s", name="sps")
                for e in range(NO):
                    nc.tensor.matmul(sps[:], qt[e][:, j * P:(j + 1) * P],
                                     kt[e][:, c - p * 1024: c - p * 1024 + cw],
                                     start=(e == 0), stop=(e == NO - 1))
                if final_pass and c + cw == ext * P:
                    off = cw - 2 * P
                    nc.vector.tensor_add(sps[:, off:off + 2 * P],
                                         sps[:, off:off + 2 * P], mask_sb[:])
                pch = sb_p.tile([P, cw], dt.bfloat16, tag="p", name="pch")
                ci = chunk_ctr[j]
                chunk_ctr[j] += 1
                nc.scalar.activation(pch[:], sps[:], AF.Exp, scale=SCALE,
                                     accum_out=sums[:, 4 * j + ci:4 * j + ci + 1])
                for ti in range(cw // P):
                    t = c // P + ti
                    ptp = ps.tile([P, P], dt.bfloat16, tag="tr", name="ptp")
                    nc.tensor.transpose(ptp[:], pch[:, ti * P:(ti + 1) * P], ident_b[:])
                    ptb = sb_pt.tile([P, P], dt.bfloat16, tag="pt", name="ptb")
                    nc.vector.tensor_copy(ptb[:], ptp[:])
                    for ec in range(2):
                        nc.tensor.matmul(o_ps[ec][:], ptb[:],
                                         vt[t - HT * p][:, ec * 512:(ec + 1) * 512],
                                         start=(t == t0), stop=(t == t1 - 1))
                c += cw

            if final_pass:
                ssum = sums[:, 32 + 2 * j:33 + 2 * j]
                nc.vector.reduce_sum(ssum, sums[:, 4 * j:4 * j + chunk_ctr[j]],
                                     axis=mybir.AxisListType.X)
                rec = sums[:, 33 + 2 * j:34 + 2 * j]
                nc.vector.reciprocal(rec, ssum)
                ob = sb_stage.tile([P, D], dt.float32, tag="obf", name="obfin")
                for ec in range(2):
                    if ext > HT:
                        oj = sb_ojoin.tile([P, 512], dt.float32, tag="ojoin", name="oj")
                        nc.vector.tensor_add(oj[:], o_ps[ec][:],
                                             obuf[j - 4][:, ec * 512:(ec + 1) * 512])
                        nc.scalar.mul(ob[:, ec * 512:(ec + 1) * 512], oj[:], rec)
                    else:
                        nc.scalar.mul(ob[:, ec * 512:(ec + 1) * 512], o_ps[ec][:], rec)
                nc.sync.dma_start(O[j * P:(j + 1) * P, :], ob[:])
            else:
                for ec in range(2):
                    nc.vector.tensor_copy(obuf[j - 4][:, ec * 512:(ec + 1) * 512],
                                          o_ps[ec][:])


def build_program(reps: int = 1):
    nc = bacc.Bacc("TRN2", target_bir_lowering=False, debug=False, num_devices=NCORES)

    xqT_t = nc.dram_tensor("xqT", [D, SLOTS * P], dt.float32r, kind="ExternalInput")
    Wq_t = nc.dram_tensor("Wq", [D, D], dt.float32r, kind="ExternalInput")
    PSTAT_t = nc.dram_tensor("PSTAT", [P, 2 * NO * ND * P], dt.bfloat16, kind="ExternalInput")
    PMOV_t = nc.dram_tensor("PMOV", [P, 2 * ND * D], dt.bfloat16, kind="ExternalInput")
    mask_t = nc.dram_tensor("mask", [P, 2 * P], dt.float32, kind="ExternalInput")
    ident_t = nc.dram_tensor("ident", [P, P], dt.bfloat16, kind="ExternalInput")
    O_t = nc.dram_tensor("O", [SLOTS * P, D], dt.float32, kind="ExternalOutput")

    with tile.TileContext(nc) as tc:
        with (
            tc.tile_pool(name="const", bufs=1) as sb_const,
            tc.tile_pool(name="xq", bufs=1) as sb_xq,
            tc.tile_pool(name="qt", bufs=1) as sb_qt,
            tc.tile_pool(name="kt", bufs=2) as sb_kt,
            tc.tile_pool(name="vt", bufs=2) as sb_vt,
            tc.tile_pool(name="wstat", bufs=3) as sb_wblk,
            tc.tile_pool(name="wmov", bufs=2) as sb_wmov,
            tc.tile_pool(name="stage", bufs=3) as sb_stage,
            tc.tile_pool(name="p", bufs=3) as sb_p,
            tc.tile_pool(name="pt", bufs=6) as sb_pt,
            tc.tile_pool(name="sums", bufs=1) as sb_sums,
            tc.tile_pool(name="obuf", bufs=1) as sb_obuf,
            tc.tile_pool(name="ojoin", bufs=2) as sb_ojoin,
            tc.tile_pool(name="dram", bufs=2, space=bass.MemorySpace.DRAM) as dram,
            tc.tile_pool(name="ps", bufs=2, space=bass.MemorySpace.PSUM) as ps,
        ):
            pools = (sb_const, sb_xq, sb_qt, sb_kt, sb_vt, sb_wblk, sb_wmov,
                     sb_stage, sb_p, sb_pt, sb_sums, sb_obuf, sb_ojoin, dram, ps)
            # reps are unrolled (python loop, NOT tc.For_i): collectives are
            # not supported inside hardware loops on this runtime path.
            for r in range(reps):
                aps = (xqT_t.ap(), Wq_t.ap(), PSTAT_t.ap(), PMOV_t.ap(),
                       mask_t.ap(), ident_t.ap(), O_t.ap())
                _emit_body(nc, tc, pools, aps, r)

    nc.compile()
    return nc


def round_f32r(a):
    """Round fp32 to the fp32r grid (low 12 mantissa bits dropped, nearest-even)."""
    u = np.ascontiguousarray(a, np.float32).view(np.uint32).copy()
    low = u & np.uint32(0xFFF)
    base = u & np.uint32(0xFFFFF000)
    up = (low > 0x800) | ((low == 0x800) & (((base >> 12) & 1) == 1))
    base[up] += np.uint32(0x1000)
    return base.view(np.float32)


def pack_w_blocks(W):
    """[D, D] -> [D, D]: row (e*P+p), col (d*P+c) = W[d*P+p, e*P+c]."""
    return np.ascontiguousarray(
        W.reshape(ND, P, NO, P).transpose(2, 1, 0, 3).reshape(D, D))


def make_in_maps(x, Wq, Wk, Wv):
    import ml_dtypes
    x = round_f32r(x.reshape(B, S, D))
    Wq_p = pack_w_blocks(round_f32r(Wq))
    Wk_p = pack_w_blocks(round_f32r(Wk))
    Wv_r = round_f32r(Wv)
    ident = np.eye(P, dtype=np.float32).astype(ml_dtypes.bfloat16)
    tri = np.where(np.arange(P)[None, :] <= np.arange(P)[:, None], 0.0, NEG).astype(np.float32)
    masks = [
        np.concatenate([tri, np.full((P, P), NEG, np.float32)], axis=1),   # parity 0
        np.concatenate([np.zeros((P, P), np.float32), tri], axis=1),       # parity 1
    ]
    xT = [np.ascontiguousarray(x[b].T) for b in range(B)]   # [D, S] per batch

    # K-core PSTAT: pack[o*P+p, d*P+c] -> [p, o*1024 + d*128 + c], same both passes
    pk = Wk_p.reshape(NO, P, ND * P).transpose(1, 0, 2).reshape(P, NO * ND * P)
    PSTAT_K = np.ascontiguousarray(np.concatenate([pk, pk], axis=1)).astype(ml_dtypes.bfloat16)
    # V-core PMOV: Wv[d*P+p, c] -> [p, d*1024 + c], same both passes
    pv = Wv_r.reshape(ND, P, D).transpose(1, 0, 2).reshape(P, ND * D)
    PMOV_V = np.ascontiguousarray(np.concatenate([pv, pv], axis=1)).astype(ml_dtypes.bfloat16)

    in_maps = []
    for c in range(NCORES):
        b, r = c // 2, c % 2
        xTb = xT[b]
        cols = np.concatenate([np.arange((2 * j + r) * P, (2 * j + r + 1) * P)
                               for j in range(SLOTS)])
        xqTb = np.ascontiguousarray(xTb[:, cols])
        if r == 0:
            pstat = PSTAT_K
            # PMOV: xT[d*P+p, pass*1024 + c] -> [p, pass*8192 + d*1024 + c]
            pm = xTb.reshape(ND, P, 2, D).transpose(1, 2, 0, 3)
            pmov = np.ascontiguousarray(pm.reshape(P, 2 * ND * D)).astype(ml_dtypes.bfloat16)
        else:
            # PSTAT: xT[d*P+p, pass*1024 + o*128 + c] -> [p, pass*8192 + o*1024 + d*128 + c]
            t4 = xTb.reshape(ND, P, 2, NO, P).transpose(1, 2, 3, 0, 4)
            pstat = np.ascontiguousarray(t4.reshape(P, 2 * NO * ND * P)).astype(ml_dtypes.bfloat16)
            pmov = PMOV_V
        in_maps.append({
            "xqT": xqTb, "Wq": Wq_p, "PSTAT": pstat, "PMOV": pmov,
            "mask": masks[r], "ident": ident,
        })
    return in_maps


def assemble_output(results):
    out = np.empty((B, S, D), dtype=np.float32)
    for c in range(NCORES):
        b, r = c // 2, c % 2
        oc = results[c]["O"].reshape(SLOTS, P, D)
        for j in range(SLOTS):
            out[b, (2 * j + r) * P:(2 * j + r + 1) * P, :] = oc[j]
    return out


_nc_cache = {}


def _get_program(reps: int = 1):
    if reps not in _nc_cache:
        _nc_cache[reps] = build_program(reps)
    return _nc_cache[reps]


def kernel(x, Wq, Wk, Wv):
    x = np.asarray(x, dtype=np.float32)
    Wq = np.asarray(Wq, dtype=np.float32)
    Wk = np.asarray(Wk, dtype=np.float32)
    Wv = np.asarray(Wv, dtype=np.float32)
    nc = _get_program(1)
    in_maps = make_in_maps(x, Wq, Wk, Wv)
    results = run_bass_kernel_spmd(nc, in_maps, list(range(NCORES))).results
    return assemble_output(results)
